# revision 3
# baseline (speedup 1.0000x reference)
"""RNN-T joint network (dense MLP) Trainium2 Bass kernel.

Math (per batch row n):
    h = relu(f @ W1t.T + g @ W1p.T + b1t + b1p)    # [N, 512]
    y = h @ W2.T + b2                              # [N, 29]

Strategy: data-parallel over batch N=32768 across 8 NeuronCores (4096
rows/core); weights replicated.  Host-side layout prep: x = concat(f, g)
transposed to [K, N] so contraction K sits on SBUF partitions with zero
on-device transposes; K padded 1344 -> 1408 (11 full 128-tiles).
On-device: h.T[j, n] in PSUM via 11 accumulating matmuls per j-tile
(float32r, 1 cyc/row), bias+relu via ScalarE, second matmul contracts
j into y.T[29, n], bias via ScalarE, DMA out.  Host transposes y back.
"""

import numpy as np

import concourse.bacc as bacc
import concourse.bass as bass  # noqa: F401
import concourse.mybir as mybir
from concourse import tile
from concourse.bass_utils import run_bass_kernel_spmd

TRANS_H, PRED_H, JOINT_H, NUM_LABELS = 1024, 320, 512, 29
BATCH = 32768
N_CORES = 8
N_PER_CORE = BATCH // N_CORES          # 4096
K_TOTAL = TRANS_H + PRED_H             # 1344
K_PAD = 1408                           # 11 * 128
K_TILES = K_PAD // 128                 # 11
J_TILES = JOINT_H // 128               # 4
N_CHUNK = 512                          # fp32 moving-operand / PSUM-bank limit
N_CHUNKS = N_PER_CORE // N_CHUNK       # 8

F32 = mybir.dt.float32
F32R = mybir.dt.float32r

_NC_CACHE = {}


def _build_bass():
    """Build the single-core Bass program (same NEFF runs SPMD on 8 cores)."""
    nc = bacc.Bacc(None)

    xT = nc.dram_tensor("xT", [K_PAD, N_PER_CORE], F32R, kind="ExternalInput")
    w1 = nc.dram_tensor("w1", [K_PAD, JOINT_H], F32R, kind="ExternalInput")
    b1 = nc.dram_tensor("b1", [JOINT_H, 1], F32, kind="ExternalInput")
    w2T = nc.dram_tensor("w2T", [JOINT_H, NUM_LABELS], F32R, kind="ExternalInput")
    b2 = nc.dram_tensor("b2", [NUM_LABELS, 1], F32, kind="ExternalInput")
    yT = nc.dram_tensor("yT", [NUM_LABELS, N_PER_CORE], F32, kind="ExternalOutput")

    with tile.TileContext(nc) as tc:
        with (
            tc.tile_pool(name="consts", bufs=1) as consts,
            tc.tile_pool(name="xpool", bufs=2) as xpool,
            tc.tile_pool(name="hpool", bufs=2) as hpool,
            tc.tile_pool(name="opool", bufs=2) as opool,
            tc.tile_pool(name="psum_h", bufs=6, space="PSUM") as psum_h,
            tc.tile_pool(name="psum_y", bufs=2, space="PSUM") as psum_y,
        ):
            # ---- replicated constants, loaded once ----
            w1_tiles = []
            for k in range(K_TILES):
                w1t_sb = consts.tile([128, JOINT_H], F32R, name=f"w1_{k}", tag=f"w1_{k}")
                nc.sync.dma_start(out=w1t_sb, in_=w1[k * 128:(k + 1) * 128, :])
                w1_tiles.append(w1t_sb)
            w2_tiles = []
            for j in range(J_TILES):
                w2t_sb = consts.tile([128, NUM_LABELS], F32R, name=f"w2_{j}", tag=f"w2_{j}")
                nc.sync.dma_start(out=w2t_sb, in_=w2T[j * 128:(j + 1) * 128, :])
                w2_tiles.append(w2t_sb)
            b1_tiles = []
            for j in range(J_TILES):
                b1t_sb = consts.tile([128, 1], F32, name=f"b1_{j}", tag=f"b1_{j}")
                nc.sync.dma_start(out=b1t_sb, in_=b1[j * 128:(j + 1) * 128, :])
                b1_tiles.append(b1t_sb)
            b2_sb = consts.tile([NUM_LABELS, 1], F32, name="b2_sb", tag="b2")
            nc.sync.dma_start(out=b2_sb, in_=b2[:, :])

            # ---- main loop over batch chunks of 512 ----
            for c in range(N_CHUNKS):
                n0 = c * N_CHUNK
                x_tiles = []
                for k in range(K_TILES):
                    x_sb = xpool.tile([128, N_CHUNK], F32R, name=f"x_{k}", tag=f"x_{k}")
                    nc.sync.dma_start(
                        out=x_sb, in_=xT[k * 128:(k + 1) * 128, n0:n0 + N_CHUNK]
                    )
                    x_tiles.append(x_sb)

                h_tiles = []
                for j in range(J_TILES):
                    ph = psum_h.tile([128, N_CHUNK], F32, name=f"ph_{j}", tag="ph")
                    for k in range(K_TILES):
                        nc.tensor.matmul(
                            ph,
                            lhsT=w1_tiles[k][:, j * 128:(j + 1) * 128],
                            rhs=x_tiles[k],
                            start=(k == 0),
                            stop=(k == K_TILES - 1),
                        )
                    h_sb = hpool.tile([128, N_CHUNK], F32R, name=f"h_{j}", tag=f"h_{j}")
                    nc.scalar.activation(
                        h_sb, ph, mybir.ActivationFunctionType.Relu, bias=b1_tiles[j]
                    )
                    h_tiles.append(h_sb)

                py = psum_y.tile([NUM_LABELS, N_CHUNK], F32, name="py", tag="py")
                for j in range(J_TILES):
                    nc.tensor.matmul(
                        py,
                        lhsT=w2_tiles[j],
                        rhs=h_tiles[j],
                        start=(j == 0),
                        stop=(j == J_TILES - 1),
                    )
                y_sb = opool.tile([NUM_LABELS, N_CHUNK], F32, name="y_sb", tag="y")
                nc.scalar.activation(
                    y_sb, py, mybir.ActivationFunctionType.Identity, bias=b2_sb
                )
                nc.sync.dma_start(out=yT[:, n0:n0 + N_CHUNK], in_=y_sb)

    nc.finalize()
    return nc


def _get_nc():
    if "nc" not in _NC_CACHE:
        _NC_CACHE["nc"] = _build_bass()
    return _NC_CACHE["nc"]


def _prep_in_maps(f, g, W1t, b1t, W1p, b1p, W2, b2):
    f2 = np.asarray(f, np.float32).reshape(BATCH, TRANS_H)
    g2 = np.asarray(g, np.float32).reshape(BATCH, PRED_H)

    w1 = np.zeros((K_PAD, JOINT_H), np.float32)
    w1[:TRANS_H] = np.asarray(W1t, np.float32).T
    w1[TRANS_H:K_TOTAL] = np.asarray(W1p, np.float32).T
    b1 = (np.asarray(b1t, np.float32) + np.asarray(b1p, np.float32)).reshape(
        JOINT_H, 1
    )
    w2T = np.ascontiguousarray(np.asarray(W2, np.float32).T)
    b2c = np.asarray(b2, np.float32).reshape(NUM_LABELS, 1)

    in_maps = []
    for core in range(N_CORES):
        sl = slice(core * N_PER_CORE, (core + 1) * N_PER_CORE)
        xT = np.zeros((K_PAD, N_PER_CORE), np.float32)
        xT[:TRANS_H] = f2[sl].T
        xT[TRANS_H:K_TOTAL] = g2[sl].T
        in_maps.append(
            {"xT": xT, "w1": w1, "b1": b1, "w2T": w2T, "b2": b2c}
        )
    return in_maps


def _gather(results):
    y = np.empty((1, BATCH, NUM_LABELS), np.float32)
    for core, r in enumerate(results):
        y[0, core * N_PER_CORE:(core + 1) * N_PER_CORE] = r["yT"].T
    return y


def _run(inputs, trace=False):
    in_maps = _prep_in_maps(
        inputs["f"], inputs["g"], inputs["W1t"], inputs["b1t"],
        inputs["W1p"], inputs["b1p"], inputs["W2"], inputs["b2"],
    )
    res = run_bass_kernel_spmd(
        _get_nc(), in_maps, core_ids=list(range(N_CORES)), trace=trace
    )
    return _gather(res.results), res


def kernel(**inputs) -> np.ndarray:
    out, _ = _run(inputs, trace=False)
    return out


# revision 14
# speedup vs baseline: 1.0912x; 1.0912x over previous
"""RNN-T joint network (dense MLP) Trainium2 Bass kernel.

Math (per batch row n):
    h = relu(f @ W1t.T + g @ W1p.T + b1t + b1p)    # [N, 512]
    y = h @ W2.T + b2                              # [N, 29]

Strategy: data-parallel over batch N=32768 across 8 NeuronCores (4096
rows/core); weights replicated.  Host-side layout prep: x = concat(f, g)
transposed to [K, N] so contraction K sits on SBUF partitions with zero
on-device transposes; K padded 1344 -> 1408 (11 full 128-tiles).
On-device: h.T[j, n] in PSUM via 11 accumulating matmuls per j-tile
(float32r, 1 cyc/row), bias+relu via ScalarE, second matmul contracts
j into y.T[29, n], bias via ScalarE, DMA out.  Host transposes y back.
"""

import numpy as np

import concourse.bacc as bacc
import concourse.bass as bass  # noqa: F401
import concourse.mybir as mybir
from concourse import tile
from concourse.bass_utils import run_bass_kernel_spmd

TRANS_H, PRED_H, JOINT_H, NUM_LABELS = 1024, 320, 512, 29
BATCH = 32768
N_CORES = 8
N_PER_CORE = BATCH // N_CORES          # 4096
K_TOTAL = TRANS_H + PRED_H             # 1344
K_PAD = 1408                           # 11 * 128
K_TILES = K_PAD // 128                 # 11
J_TILES = JOINT_H // 128               # 4
N_CHUNK = 512                          # fp32 moving-operand / PSUM-bank limit
N_CHUNKS = N_PER_CORE // N_CHUNK       # 8

F32 = mybir.dt.float32
F32R = mybir.dt.float32r

_NC_CACHE = {}


def _build_bass():
    """Build the single-core Bass program (same NEFF runs SPMD on 8 cores)."""
    nc = bacc.Bacc(None)

    xT = nc.dram_tensor("xT", [K_PAD, N_PER_CORE], F32R, kind="ExternalInput")
    w1 = nc.dram_tensor("w1", [K_PAD, JOINT_H], F32R, kind="ExternalInput")
    b1 = nc.dram_tensor("b1", [JOINT_H, 1], F32, kind="ExternalInput")
    w2T = nc.dram_tensor("w2T", [JOINT_H, NUM_LABELS], F32R, kind="ExternalInput")
    b2 = nc.dram_tensor("b2", [NUM_LABELS, 1], F32, kind="ExternalInput")
    yT = nc.dram_tensor("yT", [NUM_LABELS, N_PER_CORE], F32, kind="ExternalOutput")

    # views with the k-tile index explicit: row (k*128 + p) -> [p, k, ...]
    xT3 = xT.rearrange("(k p) n -> p k n", p=128)     # [128, K_TILES, N]
    w13 = w1.rearrange("(k p) j -> p k j", p=128)     # [128, K_TILES, JOINT_H]

    # k-tile split for each x-chunk DMA (2 pieces -> pipeline fill + issue amortized)
    K_SPLITS = [(0, 6), (6, K_TILES)]
    # finer pieces for the pipeline-fill chunk so the first matmuls start early
    K_SPLITS_FILL = [(0, 2), (2, 4), (4, 6), (6, 8), (8, K_TILES)]

    with tile.TileContext(nc) as tc:
        with (
            tc.tile_pool(name="consts", bufs=1) as consts,
            tc.tile_pool(name="xpool", bufs=3) as xpool,
            tc.tile_pool(name="hpool", bufs=2) as hpool,
            tc.tile_pool(name="opool", bufs=2) as opool,
            tc.tile_pool(name="psum_h", bufs=6, space="PSUM") as psum_h,
            tc.tile_pool(name="psum_y", bufs=2, space="PSUM") as psum_y,
        ):
            # ---- replicated constants (ACT-ring DMAs; x rides the SP ring) ----
            w1_sb = consts.tile([128, K_TILES, JOINT_H], F32R, name="w1_sb", tag="w1")
            for (ka, kb) in K_SPLITS_FILL:
                nc.scalar.dma_start(out=w1_sb[:, ka:kb, :], in_=w13[:, ka:kb, :])
            w2_sb = consts.tile([128, J_TILES, NUM_LABELS], F32R, name="w2_sb", tag="w2")
            nc.scalar.dma_start(
                out=w2_sb,
                in_=w2T.rearrange("(j p) l -> p j l", p=128),
            )
            b1_sb = consts.tile([128, J_TILES], F32, name="b1_sb", tag="b1")
            nc.scalar.dma_start(
                out=b1_sb, in_=b1.rearrange("(j p) o -> p (j o)", p=128)
            )
            b2_sb = consts.tile([NUM_LABELS, 1], F32, name="b2_sb", tag="b2")
            nc.scalar.dma_start(out=b2_sb, in_=b2[:, :])

            # ---- main loop over batch chunks of 512 ----
            for c in range(N_CHUNKS):
                n0 = c * N_CHUNK
                x_sb = xpool.tile([128, K_TILES, N_CHUNK], F32R, name="x_sb", tag="x")
                for (ka, kb) in (K_SPLITS_FILL if c <= 2 else K_SPLITS):
                    nc.sync.dma_start(
                        out=x_sb[:, ka:kb, :], in_=xT3[:, ka:kb, n0:n0 + N_CHUNK]
                    )

                h_tiles = []
                for j in range(J_TILES):
                    ph = psum_h.tile([128, N_CHUNK], F32, name=f"ph_{j}", tag="ph")
                    for k in range(K_TILES):
                        nc.tensor.matmul(
                            ph,
                            lhsT=w1_sb[:, k, j * 128:(j + 1) * 128],
                            rhs=x_sb[:, k, :],
                            start=(k == 0),
                            stop=(k == K_TILES - 1),
                        )
                    h_sb = hpool.tile([128, N_CHUNK], F32R, name=f"h_{j}", tag=f"h_{j}")
                    nc.scalar.activation(
                        h_sb, ph, mybir.ActivationFunctionType.Relu,
                        bias=b1_sb[:, j:j + 1],
                    )
                    h_tiles.append(h_sb)

                py = psum_y.tile([NUM_LABELS, N_CHUNK], F32, name="py", tag="py")
                for j in range(J_TILES):
                    nc.tensor.matmul(
                        py,
                        lhsT=w2_sb[:, j, :],
                        rhs=h_tiles[j],
                        start=(j == 0),
                        stop=(j == J_TILES - 1),
                    )
                y_sb = opool.tile([NUM_LABELS, N_CHUNK], F32, name="y_sb", tag="y")
                nc.scalar.activation(
                    y_sb, py, mybir.ActivationFunctionType.Identity, bias=b2_sb
                )
                nc.scalar.dma_start(out=yT[:, n0:n0 + N_CHUNK], in_=y_sb)

    nc.finalize()
    return nc


def _get_nc():
    if "nc" not in _NC_CACHE:
        _NC_CACHE["nc"] = _build_bass()
    return _NC_CACHE["nc"]


def _prep_in_maps(f, g, W1t, b1t, W1p, b1p, W2, b2):
    f2 = np.asarray(f, np.float32).reshape(BATCH, TRANS_H)
    g2 = np.asarray(g, np.float32).reshape(BATCH, PRED_H)

    w1 = np.zeros((K_PAD, JOINT_H), np.float32)
    w1[:TRANS_H] = np.asarray(W1t, np.float32).T
    w1[TRANS_H:K_TOTAL] = np.asarray(W1p, np.float32).T
    b1 = (np.asarray(b1t, np.float32) + np.asarray(b1p, np.float32)).reshape(
        JOINT_H, 1
    )
    w2T = np.ascontiguousarray(np.asarray(W2, np.float32).T)
    b2c = np.asarray(b2, np.float32).reshape(NUM_LABELS, 1)

    in_maps = []
    for core in range(N_CORES):
        sl = slice(core * N_PER_CORE, (core + 1) * N_PER_CORE)
        xT = np.zeros((K_PAD, N_PER_CORE), np.float32)
        xT[:TRANS_H] = f2[sl].T
        xT[TRANS_H:K_TOTAL] = g2[sl].T
        in_maps.append(
            {"xT": xT, "w1": w1, "b1": b1, "w2T": w2T, "b2": b2c}
        )
    return in_maps


def _gather(results):
    y = np.empty((1, BATCH, NUM_LABELS), np.float32)
    for core, r in enumerate(results):
        y[0, core * N_PER_CORE:(core + 1) * N_PER_CORE] = r["yT"].T
    return y


def _run(inputs, trace=False):
    in_maps = _prep_in_maps(
        inputs["f"], inputs["g"], inputs["W1t"], inputs["b1t"],
        inputs["W1p"], inputs["b1p"], inputs["W2"], inputs["b2"],
    )
    res = run_bass_kernel_spmd(
        _get_nc(), in_maps, core_ids=list(range(N_CORES)), trace=trace
    )
    return _gather(res.results), res


def kernel(**inputs) -> np.ndarray:
    out, _ = _run(inputs, trace=False)
    return out
